# revision 3
# baseline (speedup 1.0000x reference)
"""CharEmb kernel for Trainium2 (8 NeuronCores, batch-sharded). v2.

Computation (per word of 32 chars):
  emb = table[ids]                  # [32 chars, 64] per word
  x[i, j] = emb[i//2, 32*(i%2)+j]   # raw-buffer reshape [64, 32]
  y[f, t] = sum_{i,k} x[i, t+k] * w[f, i, k]   (valid conv, K=3)
  out[f] = max_t y[f, t] + b[f]

v2 design (vs v1):
  * one-hot is precomputed on host and DMA-streamed (bf16 [101, chars]),
    natural char order -> kills gpsimd partition_broadcast + DVE is_equal.
  * gather: PE matmuls tab[101,64] x oh[101,512] -> PSUM[128,512]/chunk
    (chars 0:512 on rows 0:64, 512:1024 on rows 64:128).
  * Act copies PSUM -> SBUF bf16, DVE runs native bf16 32x32 stream
    transposes: T[32h+c, 32w+j] = emb_w[c, 32h+j]  (j contiguous).
  * shift-dup rows 64:127 = rows 0:63 shifted +1 bf16 col (tap k=1),
    one DMA per chunk pair (t_a/t_b are halves of one [128,2048] tile).
  * conv per half: fused taps k=0,1 (128-row contraction) + tap k=2
    (64 rows); moving operand stride-1 inner (t window).
  * maxpool over t=30: mixed per-chunk strategy to balance engines:
      'a' = Act copy PSUM->SBUF bf16 + DVE 5-level max tree
      'g' = Act copy + gpsimd 16-wide level-1 + DVE 4-level tree
      'd' = DVE tensor_reduce direct from PSUM
  * final: DVE bias add + one output DMA.
"""

import sys
from contextlib import ExitStack

import numpy as np

if "/opt/trn_rl_repo" not in sys.path:
    sys.path.insert(0, "/opt/trn_rl_repo")

import concourse.bass as bass
import concourse.tile as tile
from concourse import bacc, mybir
from concourse.bass_utils import run_bass_kernel_spmd

# Problem constants (hardcoded per spec)
B, S, C = 32, 512, 32
V, E = 101, 64
F, K = 128, 3
T = C - K + 1  # 30 valid conv positions
NCORES = 8
WORDS = (B * S) // NCORES  # 2048 words per core
NCHARS = WORDS * C  # 65536

CHUNK_W = 32                 # words per chunk
CH_COLS = CHUNK_W * C        # 1024 chars per chunk
NCHUNKS = WORDS // CHUNK_W   # 64
HALF_W = 16                  # words per T-tile half
HALF_COLS = HALF_W * C       # 512

f32 = mybir.dt.float32
bf16 = mybir.dt.bfloat16

# maxpool strategy per chunk (cycled): 'a' Act+DVE tree, 'g' +gpsimd L1,
# 'd' direct DVE reduce from PSUM
POOL_PATTERN = "a"


def build_kernel(num_devices=NCORES):
    nc = bacc.Bacc(
        "TRN2",
        target_bir_lowering=False,
        debug=False,
        enable_asserts=True,
        num_devices=num_devices,
    )

    oh_d = nc.dram_tensor("oh", [V, NCHARS], bf16, kind="ExternalInput")
    tab_d = nc.dram_tensor("tab", [V, E], bf16, kind="ExternalInput")
    w_d = nc.dram_tensor("wmat", [128, 256], bf16, kind="ExternalInput")
    b_d = nc.dram_tensor("bias", [128, 1], f32, kind="ExternalInput")
    out_d = nc.dram_tensor("out", [128, WORDS], f32, kind="ExternalOutput")

    with tile.TileContext(nc) as tc, ExitStack() as ctx:
        const_pool = ctx.enter_context(tc.tile_pool(name="const", bufs=1))
        oh_pool = ctx.enter_context(tc.tile_pool(name="oh", bufs=3))
        gsb_pool = ctx.enter_context(tc.tile_pool(name="gsb", bufs=3))
        tt_pool = ctx.enter_context(tc.tile_pool(name="tt", bufs=2))
        ysb_pool = ctx.enter_context(tc.tile_pool(name="ysb", bufs=3))
        y2_pool = ctx.enter_context(tc.tile_pool(name="y2", bufs=3))
        g_psum = ctx.enter_context(tc.tile_pool(name="gps", bufs=3, space="PSUM"))
        y_psum = ctx.enter_context(tc.tile_pool(name="yps", bufs=2, space="PSUM"))

        tab_sb = const_pool.tile([V, E], bf16)
        w_sb = const_pool.tile([128, 256], bf16)
        b_sb = const_pool.tile([128, 1], f32)
        obuf = const_pool.tile([128, WORDS], f32)

        nc.sync.dma_start(tab_sb[:], tab_d.ap())
        nc.sync.dma_start(w_sb[:], w_d.ap())
        nc.sync.dma_start(b_sb[:], b_d.ap())

        for pp in range(NCHUNKS // 2):  # chunk pairs
            # one-hot for the pair from DRAM: [101, 2048] bf16
            oh_t = oh_pool.tile([V, 2 * CH_COLS], bf16)
            nc.sync.dma_start(
                oh_t[:, :],
                oh_d.ap()[:, pp * 2 * CH_COLS:(pp + 1) * 2 * CH_COLS],
            )

            # T tile for the pair: cols 0:1024 = t_a (half r=0 of both
            # chunks), cols 1024:2048 = t_b
            tt = tt_pool.tile([128, 4 * HALF_COLS], bf16)

            g_list = []
            for cp in range(2):
                # gather matmuls -> PSUM [128, 512] (char halves stacked)
                g_ps = g_psum.tile([128, HALF_COLS], f32)
                for hh in range(2):
                    nc.tensor.matmul(
                        g_ps[64 * hh:64 * (hh + 1), :],
                        tab_sb[0:V, 0:E],
                        oh_t[0:V, cp * CH_COLS + 512 * hh:
                             cp * CH_COLS + 512 * (hh + 1)],
                        start=True,
                        stop=True,
                    )
                g_list.append(g_ps)

            for cp in range(2):
                g_ps = g_list[cp]
                # PSUM -> SBUF bf16
                gsb = gsb_pool.tile([128, HALF_COLS], bf16)
                nc.scalar.copy(gsb[:, :], g_ps[:, :])
                # native bf16 32x32 block transposes:
                # t_a half (words 0:15 of cp) from gsb[0:64]
                nc.vector.transpose(
                    tt[0:64, 512 * cp:512 * (cp + 1)],
                    gsb[0:64, :],
                )
                # t_b half (words 16:31 of cp) from gsb[64:128]
                nc.vector.transpose(
                    tt[0:64, 1024 + 512 * cp:1024 + 512 * (cp + 1)],
                    gsb[64:128, :],
                )

            # shift-dup rows 64:127 = rows 0:63 shifted +1 bf16 col
            # (tap k=1); one DMA covering both t_a and t_b blocks
            nc.sync.dma_start(
                tt[64:128, :].rearrange("q (r x) -> q r x", r=2)[:, :, 0:1023],
                tt[0:64, :].rearrange("q (r x) -> q r x", r=2)[:, :, 1:1024],
            )

            # conv; W loads k-outer across the pair: W01 x4 then W2 x4
            y_list = []
            views = []
            for cp in range(2):
                y_ps = y_psum.tile([128, 2 * 512], f32)
                y_list.append(y_ps)
                for r in range(2):
                    tf = (
                        tt[:, 1024 * r + 512 * cp:1024 * r + 512 * (cp + 1)]
                        .rearrange("q (w j) -> q w j", j=C)
                    )
                    tl = (
                        tt[0:64, 1024 * r + 512 * cp:1024 * r + 512 * (cp + 1)]
                        .rearrange("q (w j) -> q w j", j=C)
                    )
                    out_ap = (
                        y_ps[:, 512 * r:512 * r + HALF_W * T]
                        .rearrange("f (w t) -> f w t", t=T)
                    )
                    views.append((out_ap, tf, tl))
            for (out_ap, tf, tl) in views:
                nc.tensor.matmul(
                    out_ap, w_sb[:, 0:128], tf[:, :, 0:T],
                    start=True, stop=False, skip_group_check=True,
                )
            for (out_ap, tf, tl) in views:
                nc.tensor.matmul(
                    out_ap, w_sb[0:64, 128:256], tl[:, :, 2:2 + T],
                    start=False, stop=True, skip_group_check=True,
                )

            # maxpool over t -> obuf
            for cp in range(2):
                cc = 2 * pp + cp
                y_ps = y_list[cp]
                strat = POOL_PATTERN[cc % len(POOL_PATTERN)]
                red_out = (
                    obuf[:, cc * CHUNK_W:(cc + 1) * CHUNK_W]
                    .rearrange("f (g o) -> f g o", o=1)
                )
                if strat == "d":
                    # direct DVE reduce from PSUM
                    red_in = (
                        y_ps[:, :].rearrange("f (r x) -> f r x", x=512)
                        [:, :, 0:HALF_W * T]
                        .rearrange("f r (w t) -> f r w t", t=T)
                    )
                    nc.vector.tensor_reduce(
                        obuf[:, cc * CHUNK_W:(cc + 1) * CHUNK_W]
                        .rearrange("f (r w) -> f r w", w=HALF_W),
                        red_in, axis=mybir.AxisListType.X,
                        op=mybir.AluOpType.max,
                    )
                else:
                    # Act copies used 960 cols -> compact SBUF bf16
                    ysb = ysb_pool.tile([128, 2 * HALF_W * T], bf16)
                    nc.scalar.copy(
                        ysb[:, :].rearrange("f (r y) -> f r y", r=2),
                        y_ps[:, :].rearrange("f (r x) -> f r x", x=512)
                        [:, :, 0:HALF_W * T],
                    )
                    yv = ysb[:, :].rearrange("f (g t) -> f g t", t=T)
                    y2 = y2_pool.tile([128, 32 * 16], bf16)
                    y2v = y2[:, :].rearrange("f (g t) -> f g t", t=16)
                    # L1: 16-wide overlap max covers t 0..29
                    eng = nc.gpsimd if strat == "g" else nc.vector
                    eng.tensor_tensor(
                        y2v[:, :, :],
                        yv[:, :, 0:16],
                        yv[:, :, 14:30],
                        op=mybir.AluOpType.max,
                    )
                    # L2..L4 on DVE
                    for lo, hi, n in ((0, 8, 8), (0, 4, 4), (0, 2, 2)):
                        nc.vector.tensor_tensor(
                            y2v[:, :, lo:lo + n],
                            y2v[:, :, lo:lo + n],
                            y2v[:, :, hi:hi + n],
                            op=mybir.AluOpType.max,
                        )
                    nc.vector.tensor_tensor(
                        red_out,
                        y2v[:, :, 0:1],
                        y2v[:, :, 1:2],
                        op=mybir.AluOpType.max,
                    )

        # bias + store
        nc.vector.tensor_scalar_add(obuf[:], obuf[:], b_sb[:, 0:1])
        nc.sync.dma_start(out_d.ap(), obuf[:])

    nc.compile()
    return nc


def host_prep(char_ids, emb_table, conv_w, conv_b, num_devices=NCORES):
    """Build per-core input maps from full inputs."""
    char_ids = np.asarray(char_ids)
    emb_table = np.asarray(emb_table, dtype=np.float32)
    conv_w = np.asarray(conv_w, dtype=np.float32)
    conv_b = np.asarray(conv_b, dtype=np.float32)

    bf = mybir.dt.np(bf16)
    tab = emb_table.astype(bf)

    # one-hot [101, total_chars] in natural char order
    ids_flat = char_ids.reshape(-1).astype(np.int32)  # [B*S*C]
    oh_all = (ids_flat[None, :] == np.arange(V, dtype=np.int32)[:, None])
    oh_all = oh_all.astype(bf)

    # W layout: q = 32h + c  ->  channel i = 2c + h
    q = np.arange(64)
    ch = 2 * (q % 32) + q // 32
    wmat = np.zeros((128, 256), dtype=np.float32)
    wmat[0:64, 0:128] = conv_w[:, ch, 0].T
    wmat[64:128, 0:128] = conv_w[:, ch, 1].T
    wmat[0:64, 128:256] = conv_w[:, ch, 2].T
    wmat = wmat.astype(bf)

    bias = conv_b.reshape(128, 1).astype(np.float32)

    in_maps = []
    for jcore in range(num_devices):
        oh_core = oh_all[:, jcore * NCHARS:(jcore + 1) * NCHARS]
        in_maps.append(
            {
                "oh": np.ascontiguousarray(oh_core),
                "tab": tab,
                "wmat": wmat,
                "bias": bias,
            }
        )
    return in_maps


def _ensure_ntff_hook():
    """The agent image's antenv lacks axon_hooks; shim it and install the
    ctypes NTFF profiling hook so trace=True yields HW exec times."""
    import types

    if "antenv.axon_hooks" in sys.modules:
        return
    mod = types.ModuleType("antenv.axon_hooks")
    _hook = [None]
    mod.get_axon_ntff_profile_hook = lambda: _hook[0]
    mod.set_axon_ntff_profile_hook = lambda h: _hook.__setitem__(0, h)
    sys.modules["antenv.axon_hooks"] = mod
    try:
        import antenv

        antenv.axon_hooks = mod
        from trn_agent_boot.trn_boot import _ntff_profile_via_ctypes

        hook = _ntff_profile_via_ctypes("/opt/axon/libaxon_pjrt.so")
        mod.set_axon_ntff_profile_hook(hook)
    except Exception as e:  # degrade to no-trace
        print(f"ntff hook install failed: {e}", file=sys.stderr)


_NC_CACHE = {}


def _get_nc():
    if "nc" not in _NC_CACHE:
        _NC_CACHE["nc"] = build_kernel()
    return _NC_CACHE["nc"]


def kernel(char_ids, emb_table, conv_w, conv_b, trace=False):
    if trace:
        _ensure_ntff_hook()
    nc = _get_nc()
    in_maps = host_prep(char_ids, emb_table, conv_w, conv_b)
    res = run_bass_kernel_spmd(
        nc, in_maps, core_ids=list(range(NCORES)), trace=trace
    )
    # out[f, word] word-linear -> [word, f]
    outs = [res.results[jc]["out"].T for jc in range(NCORES)]
    full = np.concatenate(outs, axis=0).reshape(B, S, F).astype(np.float32)
    if trace:
        return full, res
    return full
